# revision 4
# baseline (speedup 1.0000x reference)
"""DCRNN Bass/Tile kernel for 8 TRN2 NeuronCores — hardware-loop edition.

Measured cost model for this device (microbench.py): every STATIC instruction
costs ~34-150us (PE matmul call ~67us, ACT ~144us, DVE ~25us, DMA ~15us) with
engines NOT concurrent, while instructions inside a hardware loop (tc.For_i)
cost ~nothing per iteration.  The previous fully-unrolled kernel (~56k static
instructions) therefore ran at ~4.9s.  This version expresses the recurrence
as three For_i loops (e0, e1, decoder) with ~200-300 static instructions per
body and all time-varying state loop-carried in fixed SBUF tiles.

Layout: activations feature-major [feat<=128, R] with R = 32*19 = 608 valid
rows; graph diffusion = DMA-transpose to row-major 114-row tiles + PE matmul
against host-built block-diag [P1^T | P2^T] (bd12).  GEMMs split rows
(304, 304); PSUM: 6 banks for gates/cand + 2 rotating banks for mixes/proj.
Sharding: data-parallel over batch (B=256 -> 32 per core), weights replicated.
"""
import numpy as np

import concourse.bass as bass
import concourse.mybir as mybir
from concourse import bacc, tile
from concourse.bass import ds
from concourse.bass_utils import run_bass_kernel_spmd

F16 = mybir.dt.float16
F32 = mybir.dt.float32
AF = mybir.ActivationFunctionType

NCORES = 8
B, T, N, DIN, U, O = 256, 64, 19, 100, 128, 100
BC = B // NCORES          # 32 batch per core
JT = 114                  # rows per mix tile (6 batch x 19)
J = 6                     # mix tiles
R = BC * N                # 608 valid rows
RP = JT * J               # 684 = mix-padded rows
RALL = 704                # col pad for DMA-transpose 128-wide windows
ROWSPLIT = ((0, 304), (304, 304))

CELLS = ("e0", "e1", "d0", "d1")
CELL_DIN = {"e0": DIN, "e1": U, "d0": DIN, "d1": U}


def _pack_layouts():
    p16, off = [], 0

    def add16(name, p, shape):
        nonlocal off
        n = int(np.prod(shape))
        p16.append((name, p, tuple(shape), off))
        off += n

    add16("bd12", JT, (J, 2 * JT))
    add16("wp", U, (O,))
    for c in CELLS:
        add16(f"{c}_wx", CELL_DIN[c], (3, 384))
        add16(f"{c}_wgh", U, (3, 256))
        add16(f"{c}_wch", U, (3, 128))
    f16_total = off
    p32, off = [], 0

    def add32(name, p, shape):
        nonlocal off
        n = int(np.prod(shape))
        p32.append((name, p, tuple(shape), off))
        off += n

    for c in CELLS:
        add32(f"{c}_bg", U, (2,))
        add32(f"{c}_bc", U, (1,))
    add32("bp", O, (1,))
    return p16, f16_total, p32, off


PACK16, F16TOT, PACK32, F32TOT = _pack_layouts()


# --------------------------------------------------------------------------
# host-side weight / input preparation
# --------------------------------------------------------------------------

def _prep_host(inputs):
    f32 = np.float32
    S = np.asarray(inputs["support"], f32)
    P1 = S
    P2 = 2.0 * (S @ S) - np.eye(N, dtype=f32)

    def bd_t(P, nb):
        Z = np.zeros((JT, JT), f32)
        for b in range(nb):
            Z[b * N:(b + 1) * N, b * N:(b + 1) * N] = P.T
        return Z

    vals = {}
    bd12 = np.zeros((JT, J, 2 * JT), f32)
    for j in range(J):
        nb = 6 if j < J - 1 else BC - 6 * (J - 1)
        bd12[:, j] = np.concatenate([bd_t(P1, nb), bd_t(P2, nb)], axis=1)
    vals["bd12"] = bd12

    for c in CELLS:
        din = CELL_DIN[c]
        Wg = np.asarray(inputs[f"{c}_Wg"], f32)   # [(din+U)*3, 2U]
        Wc = np.asarray(inputs[f"{c}_Wc"], f32)   # [(din+U)*3, U]
        wx, wgh, wch = [], [], []
        for m in range(3):
            Wg_m, Wc_m = Wg[m::3], Wc[m::3]       # [(din+U), .]
            wx.append(np.concatenate([Wg_m[:din], Wc_m[:din]], axis=1))
            wgh.append(Wg_m[din:])
            wch.append(Wc_m[din:])
        vals[f"{c}_wx"] = np.stack(wx, axis=1)    # [din, 3, 384]
        vals[f"{c}_wgh"] = np.stack(wgh, axis=1)  # [U, 3, 256]
        vals[f"{c}_wch"] = np.stack(wch, axis=1)  # [U, 3, 128]
        bg = np.asarray(inputs[f"{c}_bg"], f32)
        vals[f"{c}_bg"] = np.stack([bg[:U], bg[U:]], axis=1)
        vals[f"{c}_bc"] = np.asarray(inputs[f"{c}_bc"], f32).reshape(U, 1)
    vals["wp"] = np.asarray(inputs["Wp"], f32)
    vals["bp"] = np.asarray(inputs["bp"], f32).reshape(O, 1)

    pack16 = np.zeros((128, F16TOT), np.float16)
    for name, p, shape, off in PACK16:
        n = int(np.prod(shape))
        pack16[:p, off:off + n] = vals[name].reshape(p, n).astype(np.float16)
    pack32 = np.zeros((128, F32TOT), np.float32)
    for name, p, shape, off in PACK32:
        n = int(np.prod(shape))
        pack32[:p, off:off + n] = vals[name].reshape(p, n)
    return {"wpack16": pack16, "wpack32": pack32}


DIN16 = (DIN + 15) // 16 * 16   # 112: DMA-transpose partition padding


def _prep_xenc(enc, core, t_enc):
    """per-core encoder input -> [t_enc+1, DIN16, RALL] fp16 feature-major."""
    e = np.asarray(enc[core * BC:(core + 1) * BC], np.float32)  # [BC, T, N, DIN]
    e = e[:, :t_enc]
    fm = e.transpose(1, 3, 0, 2).reshape(t_enc, DIN, R)
    out = np.zeros((t_enc + 1, DIN16, RALL), np.float16)
    out[:t_enc, :DIN, :R] = fm
    return out


# --------------------------------------------------------------------------
# program builder
# --------------------------------------------------------------------------

def build_program(t_enc=T, t_dec=T, timing_mode=False):
    nc = bacc.Bacc()
    d = {}
    if timing_mode:
        nc.dram_tensor("tin", [1, 1], F32, kind="ExternalInput")
        d["xenc"] = nc.dram_tensor("xenc", [t_enc + 1, DIN16, RALL], F16)
        d["wpack16"] = nc.dram_tensor("wpack16", [128, F16TOT], F16)
        d["wpack32"] = nc.dram_tensor("wpack32", [128, F32TOT], F32)
        d["out"] = nc.dram_tensor("out_i", [t_dec, O, R], F16)
        tout = nc.dram_tensor("tout", [1, 1], F16, kind="ExternalOutput")
    else:
        d["xenc"] = nc.dram_tensor("xenc", [t_enc + 1, DIN16, RALL], F16,
                                   kind="ExternalInput")
        d["wpack16"] = nc.dram_tensor("wpack16", [128, F16TOT], F16,
                                      kind="ExternalInput")
        d["wpack32"] = nc.dram_tensor("wpack32", [128, F32TOT], F32,
                                      kind="ExternalInput")
        d["out"] = nc.dram_tensor("out", [t_dec, O, R], F16,
                                  kind="ExternalOutput")

    with tile.TileContext(nc) as tc:
        _emit(nc, tc, d, t_enc, t_dec)
        if timing_mode:
            with tc.tile_pool(name="tp", bufs=1) as tp:
                tt = tp.tile([1, 1], F16, name="tt")
                nc.sync.dma_start(tt[:], d["out"][t_dec - 1, :1, :1])
                nc.sync.dma_start(tout[:], tt[:])
    nc.finalize()
    return nc


def _emit(nc, tc, d, t_enc, t_dec):
    import contextlib
    stack = contextlib.ExitStack()
    with stack:
        perm = stack.enter_context(tc.tile_pool(name="perm", bufs=1))
        xp = stack.enter_context(tc.tile_pool(name="xp", bufs=2))
        rmp = stack.enter_context(tc.tile_pool(name="rmp", bufs=2))
        s12w = stack.enter_context(tc.tile_pool(name="s12w", bufs=2))
        pg = stack.enter_context(tc.tile_pool(name="pg", bufs=1, space="PSUM"))
        pmix = stack.enter_context(tc.tile_pool(name="pmix", bufs=2, space="PSUM"))

        # ---- weights ----
        wp16 = perm.tile([128, F16TOT], F16, name="wp16", tag="wp16")
        wp32 = perm.tile([128, F32TOT], F32, name="wp32", tag="wp32")
        nc.sync.dma_start(wp16[:], d["wpack16"][:])
        nc.sync.dma_start(wp32[:], d["wpack32"][:])
        w = {}
        for name, p, shape, off in PACK16:
            n = int(np.prod(shape))
            ap = wp16[:p, off:off + n]
            if len(shape) > 1:
                ap = ap.rearrange("p (a b) -> p a b", a=shape[0])
            w[name] = ap
        for name, p, shape, off in PACK32:
            n = int(np.prod(shape))
            w[name] = wp32[:p, off:off + n]

        # ---- persistent state (loop-carried) ----
        h16 = {0: perm.tile([U, RALL], F16, name="h16_0", tag="h16_0"),
               1: perm.tile([U, RALL], F16, name="h16_1", tag="h16_1")}
        hT = {0: perm.tile([U, R], F32, name="hT0", tag="hT0"),
              1: perm.tile([U, R], F32, name="hT1", tag="hT1")}
        s12h = {0: perm.tile([U, 2, RP], F16, name="s12h0", tag="s12h0"),
                1: perm.tile([U, 2, RP], F16, name="s12h1", tag="s12h1")}
        proj16 = perm.tile([DIN16, RALL], F16, name="proj16", tag="proj16")
        h0seq = perm.tile([U, t_enc, R], F16, name="h0seq", tag="h0seq")
        h0cur = perm.tile([U, RALL], F16, name="h0cur", tag="h0cur")
        rh16 = perm.tile([U, RALL], F16, name="rh16", tag="rh16")
        # elementwise temporaries
        rT = perm.tile([U, R], F32, name="rT", tag="rT")
        uT = perm.tile([U, R], F32, name="uT", tag="uT")
        cT = perm.tile([U, R], F32, name="cT", tag="cT")
        tmpT = perm.tile([U, R], F32, name="tmpT", tag="tmpT")
        for tl in (h16[0], h16[1], s12h[0], s12h[1], proj16, h0cur, rh16):
            nc.gpsimd.memset(tl[:], 0.0)
        for tl in (hT[0], hT[1]):
            nc.gpsimd.memset(tl[:], 0.0)

        def mm(*a, **kw):
            return nc.tensor.matmul(*a, **kw)

        def dmat_tiles(src, feat):
            """DMA-transpose src [feat16, RALL] fp16 -> 6 row-major tiles
            [128(>=114 valid), feat].  src partition count padded to a
            multiple of 16 (DMA xbar requirement); pad rows are zero."""
            feat16 = (feat + 15) // 16 * 16
            tiles = []
            for j in range(J):
                rm = rmp.tile([128, 128], F16, name=f"rm{j}", tag=f"rm{j}")
                nc.sync.dma_start_transpose(rm[:, :feat16],
                                            src[:feat16, j * JT:j * JT + 128])
                tiles.append(rm)
            return tiles

        def mix(rm_tiles, feat, dst):
            """rm tiles -> dst [feat, 2, RP] fp16 = [X P1 | X P2] col-mixed.
            Two matmuls share a PSUM bank writing disjoint halves; start=True
            on both (start clears only the written elements)."""
            for jj in range(3):
                pm = pmix.tile([feat, 2, 2 * JT], F32, name=f"pm{jj}", tag="pm")
                for k in range(2):
                    j = 2 * jj + k
                    mm(pm[:, k, :], rm_tiles[j][:JT, :feat], w["bd12"][:, j],
                       start=True, stop=True, skip_group_check=True)
                nc.vector.tensor_copy(
                    dst[:feat, :, jj * 2 * JT:(jj + 1) * 2 * JT].rearrange(
                        "p m (k c) -> p m k c", k=2),
                    pm.rearrange("p k (m c) -> p m k c", m=2))

        def cell(cn, layer, xfm, s12x, dfeat, mix_hnew_dst):
            """One DCGRU cell step for layer state (hT/h16/s12h[layer]).
            xfm: [dfeat, .] fp16 x input (feature-major); s12x: its mix.
            mix_hnew_dst: where to write mix(h_new) (s12h[layer] or None)."""
            st = hT[layer]
            h = h16[layer]
            sh = s12h[layer]
            # psum: gates r,u and candidate c as 2-bank tiles [U, 2, 512]
            # (each 304-row half bank-aligned at 2KB so matmul outs stay
            # within a bank; the sigmoid/tanh reads both halves in one op)
            pr = pg.tile([U, 2, 512], F32, name="pr", tag="pr", bufs=1)
            pu = pg.tile([U, 2, 512], F32, name="pu", tag="pu", bufs=1)
            pc = pg.tile([U, 2, 512], F32, name="pc", tag="pc", bufs=1)
            for half, (lo, sz) in enumerate(ROWSPLIT):
                for m in range(3):
                    xr = xfm[:dfeat, lo:lo + sz] if m == 0 \
                        else s12x[:dfeat, m - 1, lo:lo + sz]
                    hr = h[:, lo:lo + sz] if m == 0 \
                        else sh[:, m - 1, lo:lo + sz]
                    for cix, pt in ((0, pr), (1, pu)):
                        mm(pt[:, half, :sz],
                           w[f"{cn}_wx"][:dfeat, m, bass.ts(cix, 128)],
                           xr, start=(m == 0), stop=False,
                           skip_group_check=True)
                        mm(pt[:, half, :sz],
                           w[f"{cn}_wgh"][:, m, bass.ts(cix, 128)],
                           hr, start=False, stop=(m == 2),
                           skip_group_check=True)
                    # candidate x-side only (h-side after r is known)
                    mm(pc[:, half, :sz], w[f"{cn}_wx"][:dfeat, m, 256:384],
                       xr, start=(m == 0), stop=False, skip_group_check=True)
            # r, u sigmoids over both halves at once
            r3 = rT.rearrange("p (h c) -> p h c", h=2)
            u3 = uT.rearrange("p (h c) -> p h c", h=2)
            nc.scalar.activation(r3, pr[:, :, :304], AF.Sigmoid,
                                 bias=w[f"{cn}_bg"][:, 0:1])
            nc.scalar.activation(u3, pu[:, :, :304], AF.Sigmoid,
                                 bias=w[f"{cn}_bg"][:, 1:2])
            # rh + its mix
            nc.vector.tensor_mul(rh16[:, :R], rT[:, :R], st[:, :R])
            s12rh = s12w.tile([U, 2, RP], F16, name="s12rh", tag="s12rh")
            mix(dmat_tiles(rh16, U), U, s12rh)
            # candidate h-side
            for half, (lo, sz) in enumerate(ROWSPLIT):
                for m in range(3):
                    hr = rh16[:, lo:lo + sz] if m == 0 \
                        else s12rh[:, m - 1, lo:lo + sz]
                    mm(pc[:, half, :sz], w[f"{cn}_wch"][:, m], hr,
                       start=False, stop=(m == 2), skip_group_check=True)
            nc.scalar.activation(cT.rearrange("p (h c) -> p h c", h=2),
                                 pc[:, :, :304], AF.Tanh,
                                 bias=w[f"{cn}_bc"][:, 0:1])
            # h' = u*h + (1-u)*c = c + u*(h-c)
            nc.vector.tensor_sub(tmpT[:], st[:], cT[:])
            nc.vector.tensor_mul(tmpT[:], uT[:], tmpT[:])
            nc.vector.tensor_add(st[:], cT[:], tmpT[:])
            nc.vector.tensor_copy(h[:, :R], st[:])
            if mix_hnew_dst is not None:
                mix(dmat_tiles(h, U), U, mix_hnew_dst)

        # ------------------------------------------------------------------
        # encoder layer 0
        # ------------------------------------------------------------------
        with tc.For_i(0, t_enc) as t:
            xc = xp.tile([DIN16, RALL], F16, name="xc", tag="xc")
            nc.sync.dma_start(xc[:], d["xenc"][ds(t, 1)])
            s12x = s12w.tile([DIN, 2, RP], F16, name="s12x", tag="s12x")
            mix(dmat_tiles(xc, DIN), DIN, s12x)
            cell("e0", 0, xc, s12x, DIN, s12h[0])
            nc.sync.dma_start(h0seq[:, ds(t, 1), :], h16[0][:, :R])

        # ------------------------------------------------------------------
        # encoder layer 1 (x = h0seq[t])
        # ------------------------------------------------------------------
        with tc.For_i(0, t_enc) as t:
            nc.sync.dma_start(h0cur[:, :R], h0seq[:, ds(t, 1), :])
            s12x = s12w.tile([U, 2, RP], F16, name="s12x1", tag="s12x")
            mix(dmat_tiles(h0cur, U), U, s12x)
            cell("e1", 1, h0cur, s12x, U, s12h[1])

        # ------------------------------------------------------------------
        # decoder (d0 state aliases layer-0 tiles, d1 layer-1; both enter
        # the loop holding the encoders' final h and mix(h))
        # ------------------------------------------------------------------
        with tc.For_i(0, t_dec) as s:
            s12p = s12w.tile([O, 2, RP], F16, name="s12p", tag="s12p")
            mix(dmat_tiles(proj16, O), O, s12p)
            cell("d0", 0, proj16, s12p, O, s12h[0])
            cell("d1", 1, h16[0], s12h[0], U, s12h[1])
            # projection
            for half, (lo, sz) in enumerate(ROWSPLIT):
                pp = pmix.tile([O, 304], F32, name=f"pp{half}", tag="pm")
                mm(pp[:, :sz], w["wp"][:], h16[1][:, lo:lo + sz])
                nc.vector.tensor_scalar_add(proj16[:O, lo:lo + sz],
                                            pp[:, :sz], w["bp"][:, 0:1])
            nc.sync.dma_start(d["out"][ds(s, 1)], proj16[:O, :R])


# --------------------------------------------------------------------------
# entry point
# --------------------------------------------------------------------------

def kernel(**inputs):
    arrs = _prep_host(inputs)
    nc = build_program(T, T)
    in_maps = []
    for core in range(NCORES):
        m = dict(arrs)
        m["xenc"] = _prep_xenc(inputs["encoder_inputs"], core, T)
        in_maps.append(m)
    res = run_bass_kernel_spmd(nc, in_maps, list(range(NCORES))).results
    outs = []
    for core in range(NCORES):
        o = np.asarray(res[core]["out"], np.float32)       # [T, O, R]
        o = o.reshape(T, O, BC, N).transpose(2, 0, 3, 1)   # [BC, T, N, O]
        outs.append(o)
    return np.ascontiguousarray(np.concatenate(outs, axis=0))


# revision 7
# speedup vs baseline: 1.1292x; 1.1292x over previous
"""DCRNN Bass/Tile kernel for 8 TRN2 NeuronCores — hardware-loop edition.

Measured cost model for this device (microbench.py): every STATIC instruction
costs ~34-150us (PE matmul call ~67us, ACT ~144us, DVE ~25us, DMA ~15us) with
engines NOT concurrent, while instructions inside a hardware loop (tc.For_i)
cost ~nothing per iteration.  The previous fully-unrolled kernel (~56k static
instructions) therefore ran at ~4.9s.  This version expresses the recurrence
as two For_i loops (encoder e0+e1 fused, decoder d0+d1+proj) with ~300 static
instructions per body and all time-varying state loop-carried in fixed SBUF
tiles; layer 1 consumes layer 0's fresh h and its just-computed diffusion mix
directly, so neither the h0 sequence nor its re-mix is ever materialized.

Layout: activations feature-major [feat<=128, R] with R = 32*19 = 608 valid
rows; graph diffusion = DMA-transpose to row-major 114-row tiles + PE matmul
against host-built block-diag [P1^T | P2^T] (bd12).  GEMMs split rows
(304, 304); PSUM: 6 banks for gates/cand + 2 rotating banks for mixes/proj.
Sharding: data-parallel over batch (B=256 -> 32 per core), weights replicated.
"""
import numpy as np

import concourse.bass as bass
import concourse.mybir as mybir
from concourse import bacc, tile
from concourse.bass import ds
from concourse.bass_utils import run_bass_kernel_spmd

F16 = mybir.dt.float16
F32 = mybir.dt.float32
AF = mybir.ActivationFunctionType

NCORES = 8
B, T, N, DIN, U, O = 256, 64, 19, 100, 128, 100
BC = B // NCORES          # 32 batch per core
JT = 114                  # rows per mix tile (6 batch x 19)
J = 6                     # mix tiles
R = BC * N                # 608 valid rows
RP = JT * J               # 684 = mix-padded rows
RALL = 704                # col pad for DMA-transpose 128-wide windows
ROWSPLIT = ((0, 304), (304, 304))

CELLS = ("e0", "e1", "d0", "d1")
CELL_DIN = {"e0": DIN, "e1": U, "d0": DIN, "d1": U}


def _pack_layouts():
    p16, off = [], 0

    def add16(name, p, shape):
        nonlocal off
        n = int(np.prod(shape))
        p16.append((name, p, tuple(shape), off))
        off += n

    add16("bd12", JT, (J, 2 * JT))
    add16("wp", U, (O,))
    for c in CELLS:
        add16(f"{c}_wx", CELL_DIN[c], (3, 384))
        add16(f"{c}_wgh", U, (3, 256))
        add16(f"{c}_wch", U, (3, 128))
    f16_total = off
    p32, off = [], 0

    def add32(name, p, shape):
        nonlocal off
        n = int(np.prod(shape))
        p32.append((name, p, tuple(shape), off))
        off += n

    for c in CELLS:
        add32(f"{c}_bg", U, (2,))
        add32(f"{c}_bc", U, (1,))
    add32("bp", O, (1,))
    return p16, f16_total, p32, off


PACK16, F16TOT, PACK32, F32TOT = _pack_layouts()


# --------------------------------------------------------------------------
# host-side weight / input preparation
# --------------------------------------------------------------------------

def _prep_host(inputs):
    f32 = np.float32
    S = np.asarray(inputs["support"], f32)
    P1 = S
    P2 = 2.0 * (S @ S) - np.eye(N, dtype=f32)

    def bd_t(P, nb):
        Z = np.zeros((JT, JT), f32)
        for b in range(nb):
            Z[b * N:(b + 1) * N, b * N:(b + 1) * N] = P.T
        return Z

    vals = {}
    bd12 = np.zeros((JT, J, 2 * JT), f32)
    for j in range(J):
        nb = 6 if j < J - 1 else BC - 6 * (J - 1)
        bd12[:, j] = np.concatenate([bd_t(P1, nb), bd_t(P2, nb)], axis=1)
    vals["bd12"] = bd12

    for c in CELLS:
        din = CELL_DIN[c]
        Wg = np.asarray(inputs[f"{c}_Wg"], f32)   # [(din+U)*3, 2U]
        Wc = np.asarray(inputs[f"{c}_Wc"], f32)   # [(din+U)*3, U]
        wx, wgh, wch = [], [], []
        for m in range(3):
            Wg_m, Wc_m = Wg[m::3], Wc[m::3]       # [(din+U), .]
            wx.append(np.concatenate([Wg_m[:din], Wc_m[:din]], axis=1))
            wgh.append(Wg_m[din:])
            wch.append(Wc_m[din:])
        vals[f"{c}_wx"] = np.stack(wx, axis=1)    # [din, 3, 384]
        vals[f"{c}_wgh"] = np.stack(wgh, axis=1)  # [U, 3, 256]
        vals[f"{c}_wch"] = np.stack(wch, axis=1)  # [U, 3, 128]
        bg = np.asarray(inputs[f"{c}_bg"], f32)
        vals[f"{c}_bg"] = np.stack([bg[:U], bg[U:]], axis=1)
        vals[f"{c}_bc"] = np.asarray(inputs[f"{c}_bc"], f32).reshape(U, 1)
    vals["wp"] = np.asarray(inputs["Wp"], f32)
    vals["bp"] = np.asarray(inputs["bp"], f32).reshape(O, 1)

    pack16 = np.zeros((128, F16TOT), np.float16)
    for name, p, shape, off in PACK16:
        n = int(np.prod(shape))
        pack16[:p, off:off + n] = vals[name].reshape(p, n).astype(np.float16)
    pack32 = np.zeros((128, F32TOT), np.float32)
    for name, p, shape, off in PACK32:
        n = int(np.prod(shape))
        pack32[:p, off:off + n] = vals[name].reshape(p, n)
    return {"wpack16": pack16, "wpack32": pack32}


DIN16 = (DIN + 15) // 16 * 16   # 112: DMA-transpose partition padding


def _prep_xenc(enc, core, t_enc):
    """per-core encoder input -> [t_enc+1, DIN16, RALL] fp16 feature-major."""
    e = np.asarray(enc[core * BC:(core + 1) * BC], np.float32)  # [BC, T, N, DIN]
    e = e[:, :t_enc]
    fm = e.transpose(1, 3, 0, 2).reshape(t_enc, DIN, R)
    out = np.zeros((t_enc + 1, DIN16, RALL), np.float16)
    out[:t_enc, :DIN, :R] = fm
    return out


# --------------------------------------------------------------------------
# program builder
# --------------------------------------------------------------------------

def build_program(t_enc=T, t_dec=T, timing_mode=False):
    nc = bacc.Bacc()
    d = {}
    if timing_mode:
        nc.dram_tensor("tin", [1, 1], F32, kind="ExternalInput")
        d["xenc"] = nc.dram_tensor("xenc", [t_enc + 1, DIN16, RALL], F16)
        d["wpack16"] = nc.dram_tensor("wpack16", [128, F16TOT], F16)
        d["wpack32"] = nc.dram_tensor("wpack32", [128, F32TOT], F32)
        d["out"] = nc.dram_tensor("out_i", [t_dec, O, R], F16)
        tout = nc.dram_tensor("tout", [1, 1], F16, kind="ExternalOutput")
    else:
        d["xenc"] = nc.dram_tensor("xenc", [t_enc + 1, DIN16, RALL], F16,
                                   kind="ExternalInput")
        d["wpack16"] = nc.dram_tensor("wpack16", [128, F16TOT], F16,
                                      kind="ExternalInput")
        d["wpack32"] = nc.dram_tensor("wpack32", [128, F32TOT], F32,
                                      kind="ExternalInput")
        d["out"] = nc.dram_tensor("out", [t_dec, O, R], F16,
                                  kind="ExternalOutput")

    with tile.TileContext(nc) as tc:
        _emit(nc, tc, d, t_enc, t_dec)
        if timing_mode:
            with tc.tile_pool(name="tp", bufs=1) as tp:
                tt = tp.tile([1, 1], F16, name="tt")
                nc.sync.dma_start(tt[:], d["out"][t_dec - 1, :1, :1])
                nc.sync.dma_start(tout[:], tt[:])
    nc.finalize()
    return nc


def _emit(nc, tc, d, t_enc, t_dec):
    import contextlib
    stack = contextlib.ExitStack()
    with stack:
        perm = stack.enter_context(tc.tile_pool(name="perm", bufs=1))
        xp = stack.enter_context(tc.tile_pool(name="xp", bufs=2))
        rmp = stack.enter_context(tc.tile_pool(name="rmp", bufs=2))
        s12w = stack.enter_context(tc.tile_pool(name="s12w", bufs=2))
        pg = stack.enter_context(tc.tile_pool(name="pg", bufs=1, space="PSUM"))
        pmix = stack.enter_context(tc.tile_pool(name="pmix", bufs=2, space="PSUM"))

        # ---- weights ----
        wp16 = perm.tile([128, F16TOT], F16, name="wp16", tag="wp16")
        wp32 = perm.tile([128, F32TOT], F32, name="wp32", tag="wp32")
        nc.sync.dma_start(wp16[:], d["wpack16"][:])
        nc.sync.dma_start(wp32[:], d["wpack32"][:])
        w = {}
        for name, p, shape, off in PACK16:
            n = int(np.prod(shape))
            ap = wp16[:p, off:off + n]
            if len(shape) > 1:
                ap = ap.rearrange("p (a b) -> p a b", a=shape[0])
            w[name] = ap
        for name, p, shape, off in PACK32:
            n = int(np.prod(shape))
            w[name] = wp32[:p, off:off + n]

        # ---- persistent state (loop-carried) ----
        h16 = {0: perm.tile([U, RALL], F16, name="h16_0", tag="h16_0"),
               1: perm.tile([U, RALL], F16, name="h16_1", tag="h16_1")}
        hT = {0: perm.tile([U, R], F32, name="hT0", tag="hT0"),
              1: perm.tile([U, R], F32, name="hT1", tag="hT1")}
        s12h = {0: perm.tile([U, 2, RP], F16, name="s12h0", tag="s12h0"),
                1: perm.tile([U, 2, RP], F16, name="s12h1", tag="s12h1")}
        proj16 = perm.tile([DIN16, RALL], F16, name="proj16", tag="proj16")
        rh16 = perm.tile([U, RALL], F16, name="rh16", tag="rh16")
        # elementwise temporaries
        rT = perm.tile([U, R], F32, name="rT", tag="rT")
        uT = perm.tile([U, R], F32, name="uT", tag="uT")
        cT = perm.tile([U, R], F32, name="cT", tag="cT")
        tmpT = perm.tile([U, R], F32, name="tmpT", tag="tmpT")
        for tl in (h16[0], h16[1], s12h[0], s12h[1], proj16, rh16):
            nc.gpsimd.memset(tl[:], 0.0)
        for tl in (hT[0], hT[1]):
            nc.gpsimd.memset(tl[:], 0.0)

        def mm(*a, **kw):
            return nc.tensor.matmul(*a, **kw)

        def dmat_tiles(src, feat):
            """DMA-transpose src [feat16, RALL] fp16 -> 6 row-major tiles
            [128(>=114 valid), feat].  src partition count padded to a
            multiple of 16 (DMA xbar requirement); pad rows are zero."""
            feat16 = (feat + 15) // 16 * 16
            tiles = []
            for j in range(J):
                rm = rmp.tile([128, 128], F16, name=f"rm{j}", tag=f"rm{j}")
                nc.sync.dma_start_transpose(rm[:, :feat16],
                                            src[:feat16, j * JT:j * JT + 128])
                tiles.append(rm)
            return tiles

        def mix(rm_tiles, feat, dst):
            """rm tiles -> dst [feat, 2, RP] fp16 = [X P1 | X P2] col-mixed.
            Two matmuls share a PSUM bank writing disjoint halves; start=True
            on both (start clears only the written elements)."""
            for jj in range(3):
                pm = pmix.tile([feat, 2, 2 * JT], F32, name=f"pm{jj}", tag="pm")
                for k in range(2):
                    j = 2 * jj + k
                    mm(pm[:, k, :], rm_tiles[j][:JT, :feat], w["bd12"][:, j],
                       start=True, stop=True, skip_group_check=True)
                nc.vector.tensor_copy(
                    dst[:feat, :, jj * 2 * JT:(jj + 1) * 2 * JT].rearrange(
                        "p m (k c) -> p m k c", k=2),
                    pm.rearrange("p k (m c) -> p m k c", m=2))

        def cell(cn, layer, xfm, s12x, dfeat, mix_hnew_dst):
            """One DCGRU cell step for layer state (hT/h16/s12h[layer]).
            xfm: [dfeat, .] fp16 x input (feature-major); s12x: its mix.
            mix_hnew_dst: where to write mix(h_new) (s12h[layer] or None)."""
            st = hT[layer]
            h = h16[layer]
            sh = s12h[layer]
            # psum: gates r,u and candidate c as 2-bank tiles [U, 2, 512]
            # (each 304-row half bank-aligned at 2KB so matmul outs stay
            # within a bank; the sigmoid/tanh reads both halves in one op)
            pr = pg.tile([U, 2, 512], F32, name="pr", tag="pr", bufs=1)
            pu = pg.tile([U, 2, 512], F32, name="pu", tag="pu", bufs=1)
            pc = pg.tile([U, 2, 512], F32, name="pc", tag="pc", bufs=1)
            for half, (lo, sz) in enumerate(ROWSPLIT):
                for m in range(3):
                    xr = xfm[:dfeat, lo:lo + sz] if m == 0 \
                        else s12x[:dfeat, m - 1, lo:lo + sz]
                    hr = h[:, lo:lo + sz] if m == 0 \
                        else sh[:, m - 1, lo:lo + sz]
                    for cix, pt in ((0, pr), (1, pu)):
                        mm(pt[:, half, :sz],
                           w[f"{cn}_wx"][:dfeat, m, bass.ts(cix, 128)],
                           xr, start=(m == 0), stop=False,
                           skip_group_check=True)
                        mm(pt[:, half, :sz],
                           w[f"{cn}_wgh"][:, m, bass.ts(cix, 128)],
                           hr, start=False, stop=(m == 2),
                           skip_group_check=True)
                    # candidate x-side only (h-side after r is known)
                    mm(pc[:, half, :sz], w[f"{cn}_wx"][:dfeat, m, 256:384],
                       xr, start=(m == 0), stop=False, skip_group_check=True)
            # r, u sigmoids over both halves at once
            r3 = rT.rearrange("p (h c) -> p h c", h=2)
            u3 = uT.rearrange("p (h c) -> p h c", h=2)
            nc.scalar.activation(r3, pr[:, :, :304], AF.Sigmoid,
                                 bias=w[f"{cn}_bg"][:, 0:1])
            nc.scalar.activation(u3, pu[:, :, :304], AF.Sigmoid,
                                 bias=w[f"{cn}_bg"][:, 1:2])
            # rh + its mix
            nc.vector.tensor_mul(rh16[:, :R], rT[:, :R], st[:, :R])
            s12rh = s12w.tile([U, 2, RP], F16, name="s12rh", tag="s12rh")
            mix(dmat_tiles(rh16, U), U, s12rh)
            # candidate h-side
            for half, (lo, sz) in enumerate(ROWSPLIT):
                for m in range(3):
                    hr = rh16[:, lo:lo + sz] if m == 0 \
                        else s12rh[:, m - 1, lo:lo + sz]
                    mm(pc[:, half, :sz], w[f"{cn}_wch"][:, m], hr,
                       start=False, stop=(m == 2), skip_group_check=True)
            nc.scalar.activation(cT.rearrange("p (h c) -> p h c", h=2),
                                 pc[:, :, :304], AF.Tanh,
                                 bias=w[f"{cn}_bc"][:, 0:1])
            # h' = u*h + (1-u)*c = c + u*(h-c)
            nc.vector.tensor_sub(tmpT[:], st[:], cT[:])
            nc.vector.tensor_mul(tmpT[:], uT[:], tmpT[:])
            nc.vector.tensor_add(st[:], cT[:], tmpT[:])
            nc.vector.tensor_copy(h[:, :R], st[:])
            if mix_hnew_dst is not None:
                mix(dmat_tiles(h, U), U, mix_hnew_dst)

        # ------------------------------------------------------------------
        # encoder: both layers in one loop body.  e1's x input is e0's fresh
        # h (h16[0]) and its diffusion mix is e0's just-written s12h[0] —
        # the same producer/consumer pattern as d0 -> d1 in the decoder.
        # ------------------------------------------------------------------
        with tc.For_i(0, t_enc) as t:
            xc = xp.tile([DIN16, RALL], F16, name="xc", tag="xc")
            nc.sync.dma_start(xc[:], d["xenc"][ds(t, 1)])
            s12x = s12w.tile([DIN, 2, RP], F16, name="s12x", tag="s12x")
            mix(dmat_tiles(xc, DIN), DIN, s12x)
            cell("e0", 0, xc, s12x, DIN, s12h[0])
            cell("e1", 1, h16[0], s12h[0], U, s12h[1])

        # ------------------------------------------------------------------
        # decoder (d0 state aliases layer-0 tiles, d1 layer-1; both enter
        # the loop holding the encoders' final h and mix(h))
        # ------------------------------------------------------------------
        with tc.For_i(0, t_dec) as s:
            s12p = s12w.tile([O, 2, RP], F16, name="s12p", tag="s12p")
            mix(dmat_tiles(proj16, O), O, s12p)
            cell("d0", 0, proj16, s12p, O, s12h[0])
            cell("d1", 1, h16[0], s12h[0], U, s12h[1])
            # projection
            for half, (lo, sz) in enumerate(ROWSPLIT):
                pp = pmix.tile([O, 304], F32, name=f"pp{half}", tag="pm")
                mm(pp[:, :sz], w["wp"][:], h16[1][:, lo:lo + sz])
                nc.vector.tensor_scalar_add(proj16[:O, lo:lo + sz],
                                            pp[:, :sz], w["bp"][:, 0:1])
            nc.sync.dma_start(d["out"][ds(s, 1)], proj16[:O, :R])


# --------------------------------------------------------------------------
# entry point
# --------------------------------------------------------------------------

def kernel(**inputs):
    arrs = _prep_host(inputs)
    nc = build_program(T, T)
    in_maps = []
    for core in range(NCORES):
        m = dict(arrs)
        m["xenc"] = _prep_xenc(inputs["encoder_inputs"], core, T)
        in_maps.append(m)
    res = run_bass_kernel_spmd(nc, in_maps, list(range(NCORES))).results
    outs = []
    for core in range(NCORES):
        o = np.asarray(res[core]["out"], np.float32)       # [T, O, R]
        o = o.reshape(T, O, BC, N).transpose(2, 0, 3, 1)   # [BC, T, N, O]
        outs.append(o)
    return np.ascontiguousarray(np.concatenate(outs, axis=0))
